# revision 2
# baseline (speedup 1.0000x reference)
"""CEMA recurrence kernel for trn2 (8 NeuronCores, batch-sharded).

Computes, per batch element b (one per core):
    h[e,n,t] = p[e,n]*x[t,e] + q[e,n]*h[e,n,t-1]
    y[t,e]   = sum_n g[e,n]*h[e,n,t]
mirroring the reference arithmetic op-for-op in fp32 (premul, scan with
(q*state)+b, then an n-ascending accumulation) so the inf/nan overflow
pattern matches.

Shapes (hardcoded): x [8, 4096, 512] f32; p,q [512,16]; gamma [512,16,1].
"""
import numpy as np

import concourse.bacc as bacc
import concourse.mybir as mybir
import concourse.tile as tile
import concourse.masks as masks
from concourse.bass_utils import run_bass_kernel_spmd

B, S, E, N = 8, 4096, 512, 16
P = 128              # partitions
G = E // P           # 4 e-groups
TB = 512             # t-block for transpose/DMA staging (4x128)
CS = 512             # scan chunk length
NCH = S // CS        # 8 scan chunks
F32 = mybir.dt.float32
MULT = mybir.AluOpType.mult
ADD = mybir.AluOpType.add

_compiled = None


def _build():
    nc = bacc.Bacc("TRN2", target_bir_lowering=False, debug=False, num_devices=B)

    x_d = nc.dram_tensor("x", [S, E], F32, kind="ExternalInput").ap()
    p_d = nc.dram_tensor("p", [E, N], F32, kind="ExternalInput").ap()
    q_d = nc.dram_tensor("q", [E, N], F32, kind="ExternalInput").ap()
    g_d = nc.dram_tensor("g", [E, N], F32, kind="ExternalInput").ap()
    y_d = nc.dram_tensor("y", [S, E], F32, kind="ExternalOutput").ap()

    # DRAM views: x rows t, cols e. t = tq*TB + tb*P + pt
    x_v = x_d.rearrange("(tq tb pt) (g pe) -> tq pt tb g pe", pt=P, tb=TB // P, pe=P)
    y_v = y_d.rearrange("(tq tb pt) e -> tq pt tb e", pt=P, tb=TB // P)

    with tile.TileContext(nc) as tc:
        with tc.tile_pool(name="const", bufs=1) as cpool, \
             tc.tile_pool(name="xT", bufs=1) as xTpool, \
             tc.tile_pool(name="ysb", bufs=1) as ypool, \
             tc.tile_pool(name="xin", bufs=3) as xinpool, \
             tc.tile_pool(name="bwork", bufs=4) as bpool, \
             tc.tile_pool(name="hwork", bufs=4) as hpool, \
             tc.tile_pool(name="yout", bufs=3) as opool, \
             tc.tile_pool(name="psin", bufs=3, space="PSUM") as psin, \
             tc.tile_pool(name="psout", bufs=3, space="PSUM") as psout:

            ident = cpool.tile([P, P], F32, tag="ident")
            masks.make_identity(nc, ident[:])

            # params: [E, N] -> per-group [128, N] tiles
            p_sb = cpool.tile([P, G * N], F32, tag="p")
            q_sb = cpool.tile([P, G * N], F32, tag="q")
            g_sb = cpool.tile([P, G * N], F32, tag="g")
            pv = p_d.rearrange("(g pe) n -> pe g n", pe=P)
            qv = q_d.rearrange("(g pe) n -> pe g n", pe=P)
            gv = g_d.rearrange("(g pe) n -> pe g n", pe=P)
            nc.sync.dma_start(p_sb[:].rearrange("pe (g n) -> pe g n", g=G), pv)
            nc.sync.dma_start(q_sb[:].rearrange("pe (g n) -> pe g n", g=G), qv)
            nc.sync.dma_start(g_sb[:].rearrange("pe (g n) -> pe g n", g=G), gv)

            # x transposed: per-group [128(e), 4096(t)]
            xT = [xTpool.tile([P, S], F32, tag=f"xT{g}", name=f"xT{g}") for g in range(G)]
            y_sb = [ypool.tile([P, S], F32, tag=f"ysb{g}", name=f"ysb{g}") for g in range(G)]

            # ---- Phase A: load + transpose x ----
            for tq in range(S // TB):
                xin = xinpool.tile([P, TB // P, E], F32, tag="xin")
                nc.sync.dma_start(xin[:], x_v[tq])
                for g in range(G):
                    ps = psin.tile([P, TB], F32, tag="psin")
                    for tb in range(TB // P):
                        nc.tensor.transpose(
                            ps[:, tb * P:(tb + 1) * P], xin[:, tb, g * P:(g + 1) * P],
                            ident[:],
                        )
                    nc.scalar.copy(xT[g][:, tq * TB:(tq + 1) * TB], ps[:])

            # ---- Phase B: premul / scan / reduce ----
            for g in range(G):
                for n in range(N):
                    pcol = p_sb[:, g * N + n: g * N + n + 1]
                    qcol = q_sb[:, g * N + n: g * N + n + 1]
                    gcol = g_sb[:, g * N + n: g * N + n + 1]
                    hprev = None
                    for c in range(NCH):
                        sl = slice(c * CS, (c + 1) * CS)
                        b = bpool.tile([P, CS], F32, tag="b")
                        nc.scalar.mul(b[:], xT[g][:, sl], pcol)
                        h = hpool.tile([P, CS], F32, tag="h")
                        init = 0.0 if c == 0 else hprev[:, CS - 1: CS]
                        nc.vector.tensor_tensor_scan(
                            h[:], qcol.broadcast_to([P, CS]), b[:], init, MULT, ADD,
                        )
                        if n == 0:
                            nc.vector.tensor_scalar_mul(y_sb[g][:, sl], h[:], gcol)
                        else:
                            nc.vector.scalar_tensor_tensor(
                                y_sb[g][:, sl], h[:], gcol, y_sb[g][:, sl], MULT, ADD,
                            )
                        hprev = h

            # ---- Phase C: transpose back + store ----
            for tq in range(S // TB):
                for tb in range(TB // P):
                    ps = psout.tile([P, E], F32, tag="psout")
                    for g in range(G):
                        nc.tensor.transpose(
                            ps[:, g * P:(g + 1) * P],
                            y_sb[g][:, tq * TB + tb * P: tq * TB + (tb + 1) * P],
                            ident[:],
                        )
                    yo = opool.tile([P, E], F32, tag="yout")
                    nc.scalar.copy(yo[:], ps[:])
                    nc.sync.dma_start(y_v[tq, :, tb], yo[:])

    nc.compile()
    return nc


def kernel(x, omega, p_coeff, q_coeff, gamma):
    global _compiled
    if _compiled is None:
        _compiled = _build()
    nc = _compiled

    x = np.ascontiguousarray(np.asarray(x, dtype=np.float32))
    p = np.ascontiguousarray(np.asarray(p_coeff, dtype=np.float32))
    q = np.ascontiguousarray(np.asarray(q_coeff, dtype=np.float32))
    g = np.ascontiguousarray(np.asarray(gamma, dtype=np.float32)[..., 0])

    in_maps = [{"x": x[b], "p": p, "q": q, "g": g} for b in range(B)]
    res = run_bass_kernel_spmd(nc, in_maps, core_ids=list(range(B)))
    return np.stack([res.results[b]["y"] for b in range(B)], axis=0)


# revision 4
# speedup vs baseline: 1.1393x; 1.1393x over previous
"""CEMA recurrence kernel for trn2 (8 NeuronCores, batch-sharded).

Computes, per batch element b (one per core):
    h[e,n,t] = p[e,n]*x[t,e] + q[e,n]*h[e,n,t-1]
    y[t,e]   = sum_n g[e,n]*h[e,n,t]
mirroring the reference arithmetic op-for-op in fp32 (premul, scan with
(q*state)+b, then an n-ascending accumulation) so the inf/nan overflow
pattern matches.

Shapes (hardcoded): x [8, 4096, 512] f32; p,q [512,16]; gamma [512,16,1].
"""
import numpy as np

import concourse.bacc as bacc
import concourse.mybir as mybir
import concourse.tile as tile
import concourse.masks as masks
from concourse.bass_utils import run_bass_kernel_spmd

B, S, E, N = 8, 4096, 512, 16
P = 128              # partitions
G = E // P           # 4 e-groups
TB = 512             # t-block for transpose/DMA staging (4x128)
CS = 2048            # scan chunk length
NCH = S // CS        # 8 scan chunks
F32 = mybir.dt.float32
MULT = mybir.AluOpType.mult
ADD = mybir.AluOpType.add

_compiled = None


def _build():
    nc = bacc.Bacc("TRN2", target_bir_lowering=False, debug=False, num_devices=B)

    x_d = nc.dram_tensor("x", [S, E], F32, kind="ExternalInput").ap()
    p_d = nc.dram_tensor("p", [E, N], F32, kind="ExternalInput").ap()
    q_d = nc.dram_tensor("q", [E, N], F32, kind="ExternalInput").ap()
    g_d = nc.dram_tensor("g", [E, N], F32, kind="ExternalInput").ap()
    y_d = nc.dram_tensor("y", [S, E], F32, kind="ExternalOutput").ap()

    # DRAM views: x rows t, cols e. t = tq*TB + tb*P + pt
    x_v = x_d.rearrange("(tq tb pt) (g pe) -> tq pt tb g pe", pt=P, tb=TB // P, pe=P)
    y_v = y_d.rearrange("(tq tb pt) e -> tq pt tb e", pt=P, tb=TB // P)

    with tile.TileContext(nc) as tc:
        with tc.tile_pool(name="const", bufs=1) as cpool, \
             tc.tile_pool(name="xT", bufs=1) as xTpool, \
             tc.tile_pool(name="ysb", bufs=1) as ypool, \
             tc.tile_pool(name="xin", bufs=2) as xinpool, \
             tc.tile_pool(name="bwork", bufs=3) as bpool, \
             tc.tile_pool(name="hwork", bufs=3) as hpool, \
             tc.tile_pool(name="yout", bufs=3) as opool, \
             tc.tile_pool(name="psin", bufs=3, space="PSUM") as psin, \
             tc.tile_pool(name="psout", bufs=3, space="PSUM") as psout:

            ident = cpool.tile([P, P], F32, tag="ident")
            masks.make_identity(nc, ident[:])

            # params: [E, N] -> per-group [128, N] tiles
            p_sb = cpool.tile([P, G * N], F32, tag="p")
            q_sb = cpool.tile([P, G * N], F32, tag="q")
            g_sb = cpool.tile([P, G * N], F32, tag="g")
            pv = p_d.rearrange("(g pe) n -> pe g n", pe=P)
            qv = q_d.rearrange("(g pe) n -> pe g n", pe=P)
            gv = g_d.rearrange("(g pe) n -> pe g n", pe=P)
            nc.sync.dma_start(p_sb[:].rearrange("pe (g n) -> pe g n", g=G), pv)
            nc.sync.dma_start(q_sb[:].rearrange("pe (g n) -> pe g n", g=G), qv)
            nc.sync.dma_start(g_sb[:].rearrange("pe (g n) -> pe g n", g=G), gv)

            # x transposed: per-group [128(e), 4096(t)]
            xT = [xTpool.tile([P, S], F32, tag=f"xT{g}", name=f"xT{g}") for g in range(G)]
            y_sb = [ypool.tile([P, S], F32, tag=f"ysb{g}", name=f"ysb{g}") for g in range(G)]

            # ---- Phase A: load + transpose x ----
            for tq in range(S // TB):
                xin = xinpool.tile([P, TB // P, E], F32, tag="xin")
                nc.sync.dma_start(xin[:], x_v[tq])
                for g in range(G):
                    ps = psin.tile([P, TB], F32, tag="psin")
                    for tb in range(TB // P):
                        nc.tensor.transpose(
                            ps[:, tb * P:(tb + 1) * P], xin[:, tb, g * P:(g + 1) * P],
                            ident[:],
                        )
                    nc.scalar.copy(xT[g][:, tq * TB:(tq + 1) * TB], ps[:])

            # ---- Phase B: premul / scan / reduce ----
            for g in range(G):
                for n in range(N):
                    pcol = p_sb[:, g * N + n: g * N + n + 1]
                    qcol = q_sb[:, g * N + n: g * N + n + 1]
                    gcol = g_sb[:, g * N + n: g * N + n + 1]
                    hprev = None
                    for c in range(NCH):
                        sl = slice(c * CS, (c + 1) * CS)
                        b = bpool.tile([P, CS], F32, tag="b")
                        nc.scalar.mul(b[:], xT[g][:, sl], pcol)
                        h = hpool.tile([P, CS], F32, tag="h")
                        init = 0.0 if c == 0 else hprev[:, CS - 1: CS]
                        nc.vector.tensor_tensor_scan(
                            h[:], qcol.broadcast_to([P, CS]), b[:], init, MULT, ADD,
                        )
                        if n == 0:
                            nc.vector.tensor_scalar_mul(y_sb[g][:, sl], h[:], gcol)
                        else:
                            nc.vector.scalar_tensor_tensor(
                                y_sb[g][:, sl], h[:], gcol, y_sb[g][:, sl], MULT, ADD,
                            )
                        hprev = h

            # ---- Phase C: transpose back + store ----
            for tq in range(S // TB):
                for tb in range(TB // P):
                    ps = psout.tile([P, E], F32, tag="psout")
                    for g in range(G):
                        nc.tensor.transpose(
                            ps[:, g * P:(g + 1) * P],
                            y_sb[g][:, tq * TB + tb * P: tq * TB + (tb + 1) * P],
                            ident[:],
                        )
                    yo = opool.tile([P, E], F32, tag="yout")
                    nc.scalar.copy(yo[:], ps[:])
                    nc.sync.dma_start(y_v[tq, :, tb], yo[:])

    nc.compile()
    return nc


def kernel(x, omega, p_coeff, q_coeff, gamma):
    global _compiled
    if _compiled is None:
        _compiled = _build()
    nc = _compiled

    x = np.ascontiguousarray(np.asarray(x, dtype=np.float32))
    p = np.ascontiguousarray(np.asarray(p_coeff, dtype=np.float32))
    q = np.ascontiguousarray(np.asarray(q_coeff, dtype=np.float32))
    g = np.ascontiguousarray(np.asarray(gamma, dtype=np.float32)[..., 0])

    in_maps = [{"x": x[b], "p": p, "q": q, "g": g} for b in range(B)]
    res = run_bass_kernel_spmd(nc, in_maps, core_ids=list(range(B)))
    return np.stack([res.results[b]["y"] for b in range(B)], axis=0)


# revision 6
# speedup vs baseline: 1.6262x; 1.4274x over previous
"""CEMA recurrence kernel for trn2 (8 NeuronCores, batch-sharded).

Per batch element b (one per core):
    h[e,n,t] = p[e,n]*x[t,e] + q[e,n]*h[e,n,t-1]
    y[t,e]   = sum_n g[e,n]*h[e,n,t]

Implementation: for each (e-group, n), DVE computes the recurrence in
stride-2 form via a hand-authored fused DVE op (see cema_op):
    B[c] = b[c] + q*b[c-1]            (stt prep; b = p*x from ACT)
    s[c] = q^2*s[c-2] + B[c]          (fused op, 1 elem/cycle)
    y[c] += g*s[c]                    (same fused op, downstream stages)
s == h in exact arithmetic; fp rounding differs from the sequential
reference only at the ulp level.

Shapes (hardcoded): x [8, 4096, 512] f32; p,q [512,16]; gamma [512,16,1].
"""
import numpy as np

import concourse.bacc as bacc
import concourse.mybir as mybir
import concourse.tile as tile
import concourse.masks as masks
from concourse.bass_utils import run_bass_kernel_spmd

from cema_op import register as _register_cema

B, S, E, N = 8, 4096, 512, 16
P = 128              # partitions
G = E // P           # 4 e-groups
TB = 512             # t-block for transpose/DMA staging (4x128)
F32 = mybir.dt.float32
MULT = mybir.AluOpType.mult
ADD = mybir.AluOpType.add

_compiled = None


def _build():
    cema = _register_cema()
    nc = bacc.Bacc("TRN2", target_bir_lowering=False, debug=False, num_devices=B)

    x_d = nc.dram_tensor("x", [S, E], F32, kind="ExternalInput").ap()
    p_d = nc.dram_tensor("p", [E, N], F32, kind="ExternalInput").ap()
    q_d = nc.dram_tensor("q", [E, N], F32, kind="ExternalInput").ap()
    q2_d = nc.dram_tensor("q2", [E, N], F32, kind="ExternalInput").ap()
    g_d = nc.dram_tensor("g", [E, N], F32, kind="ExternalInput").ap()
    y_d = nc.dram_tensor("y", [S, E], F32, kind="ExternalOutput").ap()

    # DRAM views: x rows t, cols e. t = tq*TB + tb*P + pt
    x_v = x_d.rearrange("(tq tb pt) (g pe) -> tq pt tb g pe", pt=P, tb=TB // P, pe=P)
    # y written per (g, tq) as [TB rows, 128-col stripe]
    y_v = y_d.rearrange("(tq t) (g pe) -> tq g t pe", t=TB, pe=P)

    with tile.TileContext(nc) as tc:
        with tc.tile_pool(name="const", bufs=1) as cpool, \
             tc.tile_pool(name="xT", bufs=1) as xTpool, \
             tc.tile_pool(name="ysb", bufs=2) as ypool, \
             tc.tile_pool(name="xin", bufs=2) as xinpool, \
             tc.tile_pool(name="bwork", bufs=2) as bpool, \
             tc.tile_pool(name="Bwork", bufs=2) as Bpool, \
             tc.tile_pool(name="yout", bufs=3) as opool, \
             tc.tile_pool(name="psin", bufs=3, space="PSUM") as psin, \
             tc.tile_pool(name="psout", bufs=3, space="PSUM") as psout:

            ident = cpool.tile([P, P], F32, tag="ident")
            masks.make_identity(nc, ident[:])

            # params: [E, N] -> per-group [128, N] tiles packed into [128, G*N]
            p_sb = cpool.tile([P, G * N], F32, tag="p")
            q_sb = cpool.tile([P, G * N], F32, tag="q")
            q2_sb = cpool.tile([P, G * N], F32, tag="q2")
            g_sb = cpool.tile([P, G * N], F32, tag="g")
            for dram, sb in ((p_d, p_sb), (q_d, q_sb), (q2_d, q2_sb), (g_d, g_sb)):
                nc.sync.dma_start(
                    sb[:].rearrange("pe (g n) -> pe g n", g=G),
                    dram.rearrange("(g pe) n -> pe g n", pe=P),
                )

            xT = [xTpool.tile([P, S], F32, tag=f"xT{g}", name=f"xT{g}") for g in range(G)]

            # ---- Phase A: load + transpose x into xT ----
            for tq in range(S // TB):
                xin = xinpool.tile([P, TB // P, E], F32, tag="xin")
                nc.sync.dma_start(xin[:], x_v[tq])
                for g in range(G):
                    ps = psin.tile([P, TB], F32, tag="psin")
                    for tb in range(TB // P):
                        nc.tensor.transpose(
                            ps[:, tb * P:(tb + 1) * P], xin[:, tb, g * P:(g + 1) * P],
                            ident[:],
                        )
                    nc.scalar.copy(xT[g][:, tq * TB:(tq + 1) * TB], ps[:])

            # ---- Phase B + C per group ----
            for g in range(G):
                y_sb = ypool.tile([P, S], F32, tag="ysb", name=f"ysb{g}")
                nc.scalar.memzero(y_sb[:])
                for n in range(N):
                    col = slice(g * N + n, g * N + n + 1)
                    b = bpool.tile([P, S], F32, tag="b")
                    nc.scalar.mul(b[:], xT[g][:], p_sb[:, col])
                    Bt = Bpool.tile([P, S], F32, tag="Bt")
                    nc.vector.tensor_copy(Bt[:, 0:1], b[:, 0:1])
                    nc.vector.scalar_tensor_tensor(
                        Bt[:, 1:S], b[:, 0:S - 1], q_sb[:, col], b[:, 1:S],
                        MULT, ADD,
                    )
                    nc.vector._custom_dve(
                        cema, out=y_sb[:], in0=Bt[:], in1=y_sb[:],
                        s0=q2_sb[:, col], s1=g_sb[:, col],
                    )

                # ---- Phase C for this group: transpose back + store stripe ----
                for tq in range(S // TB):
                    ps = psout.tile([P, TB], F32, tag="psout")
                    for tb in range(TB // P):
                        nc.tensor.transpose(
                            ps[:, tb * P:(tb + 1) * P],
                            y_sb[:, tq * TB + tb * P: tq * TB + (tb + 1) * P],
                            ident[:],
                        )
                    yo = opool.tile([P, TB], F32, tag="yout")
                    nc.scalar.copy(yo[:], ps[:])
                    nc.sync.dma_start(
                        y_v[tq, g].rearrange("(tb pt) pe -> pt tb pe", pt=P),
                        yo[:].rearrange("pt (tb pe) -> pt tb pe", pe=P),
                    )

    nc.compile()
    return nc


def kernel(x, omega, p_coeff, q_coeff, gamma):
    global _compiled
    if _compiled is None:
        _compiled = _build()
    nc = _compiled

    x = np.ascontiguousarray(np.asarray(x, dtype=np.float32))
    p = np.ascontiguousarray(np.asarray(p_coeff, dtype=np.float32))
    q = np.ascontiguousarray(np.asarray(q_coeff, dtype=np.float32))
    q2 = np.ascontiguousarray((q * q).astype(np.float32))
    g = np.ascontiguousarray(np.asarray(gamma, dtype=np.float32)[..., 0])

    in_maps = [{"x": x[b], "p": p, "q": q, "q2": q2, "g": g} for b in range(B)]
    res = run_bass_kernel_spmd(nc, in_maps, core_ids=list(range(B)))
    return np.stack([res.results[b]["y"] for b in range(B)], axis=0)
